# revision 1
# baseline (speedup 1.0000x reference)
"""AttentionBlock (GroupNorm + 1x1 QKV + MHA + proj + residual) on 8 trn2 cores.

Sharding: core c -> (batch b = c//2, t-half = c%2). Each core computes all 4
heads for its 2048 query positions; k/v are computed over the full T=4096 from
the core's batch. No cross-core communication needed.

Layout trick: attention scores are computed TRANSPOSED, S_T[s, t] (s on
partitions), so the AV matmul needs no transposes: a[ch, t] = vT[s, ch].T @
P_T[s, t]. The softmax denominator comes from a ones-column appended to vT.
GroupNorm is folded into the QKV weights (per-channel scale/shift).
"""

import math

import numpy as np

import concourse.bass as bass
import concourse.tile as tile
from concourse import bacc, mybir
from concourse import bass_utils

F32 = mybir.dt.float32
F32R = mybir.dt.float32r
BF16 = mybir.dt.bfloat16
F16 = mybir.dt.float16

B, C, HH, WW = 4, 256, 64, 64
T = HH * WW            # 4096
NH = 4                 # heads per batch
CH = C // NH           # 64 channels per head
G = 32                 # groupnorm groups
CPG = C // G           # 8 channels per group
EPS = 1e-5
NCORES = 8
THALF = T // 2         # 2048  t-columns per core
SCALE = 1.0 / math.sqrt(math.sqrt(CH))

USE_F32R = True        # f32r (tf32-like) for the big matmuls
P_DT = BF16            # dtype of exp'd attention weights + vT (bf16 or f32)

_CACHE = {}
LAST_RESULTS = None




def _r(ap):
    return ap


def _build_program():
    nc = bacc.Bacc("TRN2", target_bir_lowering=False, debug=False)

    d_xb = nc.dram_tensor("xb", [C, T], F16, kind="ExternalInput").ap()
    d_xq = nc.dram_tensor("xq", [C, THALF], F16, kind="ExternalInput").ap()
    d_xqf = nc.dram_tensor("xqf", [C, THALF], F32, kind="ExternalInput").ap()
    d_wT = nc.dram_tensor("wT", [C, 3 * C], F32, kind="ExternalInput").ap()
    d_qkvb = nc.dram_tensor("qkvb", [3 * C, 1], F32, kind="ExternalInput").ap()
    d_vbrow = nc.dram_tensor("vbrow", [1, C], F32, kind="ExternalInput").ap()
    d_pjT = nc.dram_tensor("pjT", [C, C], F16, kind="ExternalInput").ap()
    d_pjb = nc.dram_tensor("pjb", [C, 1], F32, kind="ExternalInput").ap()
    d_gnw = nc.dram_tensor("gnw", [C, 1], F32, kind="ExternalInput").ap()
    d_gnb = nc.dram_tensor("gnb", [C, 1], F32, kind="ExternalInput").ap()
    d_gsel = nc.dram_tensor("gsel", [128, 2 * G], F32, kind="ExternalInput").ap()
    d_bsel = nc.dram_tensor("bsel", [G, C], F32, kind="ExternalInput").ap()
    d_out = nc.dram_tensor("out", [C, THALF], F32, kind="ExternalOutput").ap()

    with tile.TileContext(nc) as tc:
        with tc.tile_pool(name="data", bufs=1) as data, \
             tc.tile_pool(name="small", bufs=1) as small, \
             tc.tile_pool(name="work", bufs=3) as work:

            # ---- persistent SBUF tensors ----
            x = [data.tile([128, T], F16, tag=f"x{i}", name=f"x{i}") for i in range(2)]
            xq = [data.tile([128, THALF], F16, tag=f"xq{i}", name=f"xq{i}") for i in range(2)]
            xqf = [data.tile([128, THALF], F32, tag=f"xqf{i}", name=f"xqf{i}") for i in range(2)]
            wt = [data.tile([128, 3 * C], F32, tag=f"wt{i}", name=f"wt{i}") for i in range(2)]
            wtf = [data.tile([128, 3 * C], F16, tag=f"wtf{i}", name=f"wtf{i}") for i in range(2)]
            pjt = [data.tile([128, C], F16, tag=f"pjt{i}", name=f"pjt{i}") for i in range(2)]
            q_sb = [data.tile([128, THALF], F16, tag=f"q{i}", name=f"q{i}") for i in range(2)]
            k_sb = [data.tile([128, T], F16, tag=f"k{i}", name=f"k{i}") for i in range(2)]
            # vT: per s-chunk, per head: 64 v-columns + 1 ones-column (+3 pad)
            vt = data.tile([128, T // 128, NH, 2 * CH], P_DT, tag="vt", name="vt")
            a_sb = [data.tile([128, THALF], F16, tag=f"a{i}", name=f"a{i}") for i in range(2)]
            vbias_bc = data.tile([128, C], F32, tag="vbias_bc", name="vbias_bc")

            gnw = [small.tile([128, 1], F32, tag=f"gnw{i}", name=f"gnw{i}") for i in range(2)]
            gnb = [small.tile([128, 1], F32, tag=f"gnb{i}", name=f"gnb{i}") for i in range(2)]
            pjb = [small.tile([128, 1], F32, tag=f"pjb{i}", name=f"pjb{i}") for i in range(2)]
            qkb = [small.tile([128, 1], F32, tag=f"qkb{o}", name=f"qkb{o}") for o in range(4)]
            gsel = small.tile([128, 2 * G], F32, tag="gsel", name="gsel")
            bsel = small.tile([G, C], F32, tag="bsel", name="bsel")
            vbrow = small.tile([1, C], F32, tag="vbrow", name="vbrow")

            warm = small.tile([1, 1], F32, tag="warm", name="warm")
            nc.vector.memset(warm[:], 1.0)
            nc.scalar.activation(warm[:], warm[:], mybir.ActivationFunctionType.Ln)
            for i in range(2):
                cs = slice(128 * i, 128 * (i + 1))
                for ch8 in range(8):
                    t8 = slice(512 * ch8, 512 * (ch8 + 1))
                    nc.sync.dma_start(x[i][:, t8], d_xb[cs, t8])
                for ch2 in range(2):
                    t2 = slice(1024 * ch2, 1024 * (ch2 + 1))
                    nc.sync.dma_start(xq[i][:, t2], d_xq[cs, t2])
                nc.sync.dma_start(wt[i][:], d_wT[cs, :])
                nc.sync.dma_start(pjt[i][:], d_pjT[cs, :])
                nc.sync.dma_start(gnw[i][:], d_gnw[cs, :])
                nc.sync.dma_start(gnb[i][:], d_gnb[cs, :])
                nc.sync.dma_start(pjb[i][:], d_pjb[cs, :])
            for o in range(4):
                nc.sync.dma_start(qkb[o][:], d_qkvb[128 * o:128 * (o + 1), :])
            nc.sync.dma_start(gsel[:], d_gsel[:, :])
            nc.sync.dma_start(bsel[:], d_bsel[:, :])
            nc.sync.dma_start(vbrow[:], d_vbrow[:, :])
            for i in range(2):
                csx = slice(128 * i, 128 * (i + 1))
                nc.sync.dma_start(xqf[i][:], d_xqf[csx, :])  # residual-only, not startup-critical

            # ================= GroupNorm stats -> per-channel A, B ==========
            with tc.tile_pool(name="psQK", bufs=2, space="PSUM") as psQK, \
                 tc.tile_pool(name="psAV", bufs=2, space="PSUM") as psAV:
                psS = psAV
                psM = psAV

                A = [small.tile([128, 1], F32, tag=f"A{i}", name=f"A{i}") for i in range(2)]
                Bs = [small.tile([128, 1], F32, tag=f"B{i}", name=f"B{i}") for i in range(2)]
                pcs = [small.tile([128, 2], F32, tag=f"pcs{i}", name=f"pcs{i}") for i in range(2)]
                for i in range(2):
                    stats = work.tile([128, 8, 6], F32, tag="bnstats", name="bnstats")
                    for j in range(8):
                        nc.vector.bn_stats(stats[:, j, :], x[i][:, 512 * j:512 * (j + 1)])
                    mv = work.tile([128, 2], F32, tag="bnmv", name="bnmv")
                    nc.vector.bn_aggr(mv[:], stats[:])
                    # pcs = (mean, E[x^2]) per channel
                    nc.vector.tensor_copy(pcs[i][:, 0:1], mv[:, 0:1])
                    nc.vector.tensor_tensor(pcs[i][:, 1:2], mv[:, 0:1], mv[:, 0:1],
                                            mybir.AluOpType.mult)
                    nc.vector.tensor_tensor(pcs[i][:, 1:2], pcs[i][:, 1:2], mv[:, 1:2],
                                            mybir.AluOpType.add)

                # group stats [G, 2] = (mean_g, E[x^2]_g)
                grp_ps = psS.tile([G, 2], F32, tag="av", name="s")
                nc.tensor.matmul(grp_ps[:], gsel[:, 0:G], pcs[0][:], start=True, stop=False)
                nc.tensor.matmul(grp_ps[:], gsel[:, G:2 * G], pcs[1][:], start=False, stop=True)

                grp_sb = small.tile([G, 2], F32, tag="grp_sb", name="grp_sb")
                nc.vector.tensor_copy(grp_sb[:], grp_ps[:])
                grp2 = small.tile([G, 2], F32, tag="grp2", name="grp2")  # (mean, rstd)
                var = small.tile([G, 1], F32, tag="var", name="var")
                epst = small.tile([G, 1], F32, tag="epst", name="epst")
                nc.vector.memset(epst[:], EPS)
                nc.vector.tensor_copy(grp2[:, 0:1], grp_sb[:, 0:1])
                nc.vector.tensor_tensor(var[:], grp_sb[:, 0:1], grp_sb[:, 0:1],
                                        mybir.AluOpType.mult)
                nc.vector.tensor_tensor(var[:], grp_sb[:, 1:2], var[:],
                                        mybir.AluOpType.subtract)
                nc.scalar.activation(var[:], var[:], mybir.ActivationFunctionType.Ln,
                                     bias=epst[:])
                nc.scalar.activation(grp2[:, 1:2], var[:],
                                     mybir.ActivationFunctionType.Exp, scale=-0.5)

                # broadcast to channels; A = rstd*gn_w, B = gn_b - mean*A
                for i in range(2):
                    mb_ps = psS.tile([128, 2], F32, tag="av", name="s")
                    nc.tensor.matmul(mb_ps[:], bsel[:, 128 * i:128 * (i + 1)], grp2[:],
                                     start=True, stop=True)
                    nc.vector.tensor_tensor(A[i][:], mb_ps[:, 1:2], gnw[i][:],
                                            mybir.AluOpType.mult)
                    nc.vector.tensor_tensor(Bs[i][:], mb_ps[:, 0:1], A[i][:],
                                            mybir.AluOpType.mult)
                    nc.vector.tensor_tensor(Bs[i][:], gnb[i][:], Bs[i][:],
                                            mybir.AluOpType.subtract)
                    # folded weights
                    nc.vector.tensor_scalar_mul(wtf[i][:], wt[i][:], A[i][:])

                # folded q/k biases: qkvb[o] + sum_c wT[c,o]*B[c]
                for o in range(4):
                    b_ps = psS.tile([128, 1], F32, tag="av", name="s")
                    nc.tensor.matmul(b_ps[:], wt[0][:, 128 * o:128 * (o + 1)], Bs[0][:],
                                     start=True, stop=False)
                    nc.tensor.matmul(b_ps[:], wt[1][:, 128 * o:128 * (o + 1)], Bs[1][:],
                                     start=False, stop=True)
                    nc.vector.tensor_tensor(qkb[o][:], qkb[o][:], b_ps[:],
                                            mybir.AluOpType.add)
                # folded v bias as a row [1, C], then broadcast to 128 partitions
                vb_ps = psS.tile([1, C], F32, tag="av", name="s")
                nc.tensor.matmul(vb_ps[:], Bs[0][:], wt[0][:, 2 * C:3 * C],
                                 start=True, stop=False)
                nc.tensor.matmul(vb_ps[:], Bs[1][:], wt[1][:, 2 * C:3 * C],
                                 start=False, stop=True)
                vb_row = small.tile([1, C], F32, tag="vb_row", name="vb_row")
                nc.vector.tensor_tensor(vb_row[:], vb_ps[:], vbrow[:],
                                        mybir.AluOpType.add)
                nc.gpsimd.partition_broadcast(vbias_bc[:], vb_row[:])

                # ================= q / k / vT projections ====================
                # Emission order: q(o0,t0) -> k(o0,*) -> vT(*) -> rest, so the
                # first attention unit (pair 0) can start as early as possible.
                def proj_ps():
                    return psM.tile([128, 512], F32, tag="av", name="mm")

                def qk_pair(dst, wofs, src_t, bias, o, t):
                    # two t-tiles through one 2-bank qk-slot slice: 4 mms + 1 epilogue
                    ts2 = slice(512 * t, 512 * (t + 2))
                    ps = psQK.tile([128, 3, 512], F32, tag="qk", name="kps")[:, 0:2, :]
                    for u in range(2):
                        ts = slice(512 * (t + u), 512 * (t + u + 1))
                        nc.tensor.matmul(ps[:, u, :],
                                         wtf[0][:, wofs + 128 * o:wofs + 128 * (o + 1)],
                                         src_t[0][:, ts], start=True, stop=False)
                        nc.tensor.matmul(ps[:, u, :],
                                         wtf[1][:, wofs + 128 * o:wofs + 128 * (o + 1)],
                                         src_t[1][:, ts], start=False, stop=True)
                    nc.vector.tensor_scalar(dst[o][:, ts2].rearrange("p (u f) -> p u f", u=2),
                                            ps[:], bias[:], SCALE,
                                            mybir.AluOpType.add, mybir.AluOpType.mult)

                def vt_pair(sc):
                    # chunks sc, sc+1 through one 'av' slot: 4 mms + 1 epilogue
                    ps = proj_ps().rearrange("p (u f) -> p u f", u=2)
                    for u in range(2):
                        ss = slice(128 * (sc + u), 128 * (sc + u + 1))
                        nc.tensor.matmul(ps[:, u, :], x[0][:, ss], wtf[0][:, 2 * C:3 * C],
                                         start=True, stop=False)
                        nc.tensor.matmul(ps[:, u, :], x[1][:, ss], wtf[1][:, 2 * C:3 * C],
                                         start=False, stop=True)
                    nc.vector.tensor_tensor(
                        vt[:, sc:sc + 2, :, 0:CH],
                        ps.rearrange("p u (h c) -> p u h c", h=NH),
                        bass.AP(tensor=vbias_bc[:].tensor, offset=vbias_bc[:].offset,
                                ap=[vbias_bc[:].ap[0], [0, 2], [CH, NH], [1, CH]]),
                        mybir.AluOpType.add)

                nc.gpsimd.memset(vt[:, :, :, CH:CH + 1], 1.0)  # ones column (denominator)
                nc.gpsimd.memset(vt[:, :, :, CH + 1:2 * CH], 0.0)  # zero pad to 128 for FWL
                # all projections run upfront (mid-attention the AV accumulators
                # hold both 'av' PSUM slots, freezing any deferred production);
                # k/q pairs flow through the qk banks, vt pairs through 'av'.
                qk_pair(q_sb, 0, xq, qkb[0], 0, 0)
                for t in range(0, T // 512, 2):
                    qk_pair(k_sb, C, x, qkb[2], 0, t)
                for sc in range(0, T // 128, 2):
                    vt_pair(sc)
                # remaining production rides the unit boundaries (PSUM slots
                # rotate there); each lump is ~2 pairs, hidden under the exp
                # pipeline's double buffering
                boundary_prod = {
                    0: [lambda: qk_pair(q_sb, 0, xq, qkb[0], 0, 2)],
                    1: [lambda: qk_pair(k_sb, C, x, qkb[3], 1, 0),
                        lambda: qk_pair(k_sb, C, x, qkb[3], 1, 2)],
                    2: [lambda: qk_pair(k_sb, C, x, qkb[3], 1, 4),
                        lambda: qk_pair(k_sb, C, x, qkb[3], 1, 6)],
                    3: [lambda: qk_pair(q_sb, 0, xq, qkb[1], 1, 0),
                        lambda: qk_pair(q_sb, 0, xq, qkb[1], 1, 2)],
                }

                # ================= attention + proj ==============================
                # Head pairs (2p, 2p+1) interleaved matmul-by-matmul (disjoint PE
                # row groups -> concurrent QK). AV emission runs one exp-group
                # behind QK so the in-order PE queue never stalls on the ACT.
                NSC = T // 128                      # 32 s-chunks
                seq = []
                for sc in range(NSC):
                    seq.append((0, sc))
                    seq.append((1, sc))
                groups = [seq[g:g + 3] for g in range(0, len(seq), 3)]
                with tc.tile_pool(name="pexp", bufs=4) as pexp, \
                     tc.tile_pool(name="nrm", bufs=4) as nrm, \
                     tc.tile_pool(name="outp", bufs=3) as outp:
                    def emit_av(accs, p, grp, pe):
                        for j, (hi, sc) in enumerate(grp):
                            h = 2 * p + hi
                            nc.tensor.matmul(accs[hi][:, :], vt[:, sc, h, :],
                                             pe[:, j, :],
                                             start=(sc == 0), stop=(sc == NSC - 1))

                    def proj_steps(tb, o):
                        tbs = slice(512 * tb, 512 * (tb + 1))
                        box = {}
                        def s1():
                            box["pr"] = psM.tile([128, 512], F32, tag="av", name="pr")
                            nc.tensor.matmul(box["pr"][:], pjt[0][:, 128 * o:128 * (o + 1)],
                                             a_sb[0][:, tbs], start=True, stop=False)
                        def s2():
                            nc.tensor.matmul(box["pr"][:], pjt[1][:, 128 * o:128 * (o + 1)],
                                             a_sb[1][:, tbs], start=False, stop=True)
                        def s3():
                            res = outp.tile([128, 512], F32, tag="res", name="res")
                            box["res"] = res
                            nc.vector.tensor_scalar(res[:], box["pr"][:], pjb[o][:], None,
                                                    mybir.AluOpType.add)
                            nc.vector.tensor_tensor(res[:], res[:],
                                                    xqf[o][:, tbs],
                                                    mybir.AluOpType.add)
                            nc.sync.dma_start(d_out[128 * o:128 * (o + 1), tbs], res[:])
                        return [s1, s2, s3]

                    def proj_tb(tb):
                        for o in range(2):
                            for s in proj_steps(tb, o):
                                s()

                    def normalize(p, tb, accs):
                        tbs = slice(512 * tb, 512 * (tb + 1))
                        for hi in range(2):
                            hp = slice(64 * hi, 64 * hi + 64)
                            acc = accs[hi]
                            den = nrm.tile([1, 512], F32, tag="den", name="den")
                            nc.vector.tensor_copy(den[:], acc[CH:CH + 1, :])
                            rec = nrm.tile([1, 512], F32, tag="rec", name="rec")
                            nc.vector.reciprocal_approx_fast(rec[:], den[:])
                            bc = nrm.tile([CH, 512], F32, tag="bc", name="bc")
                            nc.gpsimd.partition_broadcast(bc[:], rec[:])
                            nc.vector.tensor_tensor(a_sb[p][hp, tbs], acc[0:CH, :],
                                                    bc[:], mybir.AluOpType.mult)

                    pend = None        # (accs, p, grp, pe) AV one group behind QK
                    fin = None         # (p, tb, accs) awaiting normalize/proj
                    units = [(p, tb) for p in range(2) for tb in range(THALF // 512)]
                    for (p, tb) in units:
                        tbs = slice(512 * tb, 512 * (tb + 1))
                        qk_t = q_sb[p]
                        kk_t = k_sb[p]
                        accs = [psAV.tile([128, 512], F32, tag="av", name="av")
                                for _ in range(2)]
                        for gi, grp in enumerate(groups):
                            qkp = psQK.tile([128, 3, 512], F32, tag="qk", name="qk")
                            for j, (hi, sc) in enumerate(grp):
                                hp = slice(64 * hi, 64 * hi + 64)
                                nc.tensor.matmul(
                                    qkp[:, j, :],
                                    kk_t[hp, 128 * sc:128 * (sc + 1)],
                                    qk_t[hp, tbs],
                                    start=True, stop=True)
                            pe = pexp.tile([128, 3, 512], P_DT, tag="pe", name="pe")
                            glen = len(grp)
                            nc.scalar.activation(pe[:, 0:glen, :], qkp[:, 0:glen, :],
                                                 mybir.ActivationFunctionType.Exp)
                            if pend is not None:
                                emit_av(*pend)
                            pend = (accs, p, grp, pe)
                            if fin is not None:
                                # previous unit's epilogue, after this unit's QK
                                # stream has started feeding the ACT
                                fp_, ftb_, faccs_ = fin
                                normalize(fp_, ftb_, faccs_)
                                if fp_ == 1:
                                    proj_tb(ftb_)
                                fin = None
                        fin = (p, tb, accs)
                        for fn_ in boundary_prod.pop(4 * p + tb, []):
                            fn_()
                    emit_av(*pend)
                    normalize(*fin)
                    proj_tb(fin[1])

    nc.compile()
    return nc


def _host_consts():
    g1 = np.zeros((128, G), dtype=np.float32)
    g2 = np.zeros((128, G), dtype=np.float32)
    for c in range(128):
        g1[c, c // CPG] = 1.0 / CPG
        g2[c, G // 2 + c // CPG] = 1.0 / CPG
    gsel = np.concatenate([g1, g2], axis=1)          # [128, 2G]
    bsel = np.zeros((G, C), dtype=np.float32)
    for c in range(C):
        bsel[c // CPG, c] = 1.0
    return gsel, bsel


def kernel(x, gn_w, gn_b, qkv_w, qkv_b, proj_w, proj_b):
    global LAST_RESULTS
    if "nc" not in _CACHE:
        _CACHE["nc"] = _build_program()
    nc = _CACHE["nc"]

    x = np.ascontiguousarray(np.asarray(x, dtype=np.float32))
    xr = x.reshape(B, C, T)
    gsel, bsel = _host_consts()
    shared = {
        "wT": np.ascontiguousarray(np.asarray(qkv_w, np.float32).T),
        "qkvb": np.asarray(qkv_b, np.float32).reshape(3 * C, 1).copy(),
        "vbrow": np.asarray(qkv_b, np.float32)[2 * C:].reshape(1, C).copy(),
        "pjT": np.ascontiguousarray(np.asarray(proj_w, np.float32).T.astype(np.float16)),
        "pjb": np.asarray(proj_b, np.float32).reshape(C, 1).copy(),
        "gnw": np.asarray(gn_w, np.float32).reshape(C, 1).copy(),
        "gnb": np.asarray(gn_b, np.float32).reshape(C, 1).copy(),
        "gsel": gsel,
        "bsel": bsel,
    }
    in_maps = []
    for c in range(NCORES):
        b, hf = c // 2, c % 2
        m = dict(shared)
        m["xb"] = np.ascontiguousarray(xr[b].astype(np.float16))
        xqs = np.ascontiguousarray(xr[b][:, hf * THALF:(hf + 1) * THALF])
        m["xq"] = xqs.astype(np.float16)
        m["xqf"] = xqs
        in_maps.append(m)

    res = bass_utils.run_bass_kernel_spmd(nc, in_maps, core_ids=list(range(NCORES)))
    LAST_RESULTS = res

    out = np.empty((B, C, T), dtype=np.float32)
    for c in range(NCORES):
        b, hf = c // 2, c % 2
        out[b][:, hf * THALF:(hf + 1) * THALF] = res.results[c]["out"]
    return out.reshape(B, C, HH, WW)



# revision 5
# speedup vs baseline: 1.0031x; 1.0031x over previous
"""AttentionBlock (GroupNorm + 1x1 QKV + MHA + proj + residual) on 8 trn2 cores.

Sharding: core c -> (batch b = c//2, t-half = c%2). Each core computes all 4
heads for its 2048 query positions; k/v are computed over the full T=4096 from
the core's batch. No cross-core communication needed.

The host ROTATES each core's [C, T] input so the core's query half is always
columns 0..2047; k/v see the rotated full T (softmax over s is order
invariant as long as k_sb and vt share the order).

Layout trick: attention scores are computed TRANSPOSED, S_T[s, t] (s on
partitions), so the AV matmul needs no transposes: a[ch, t] = vT[s, ch].T @
P_T[s, t]. The softmax denominator comes from a ones-column appended to vT.
GroupNorm is folded into the QKV weights (per-channel scale/shift).

PSUM (8 banks): qk groups-of-2 [128,2,512] x bufs2 = 4 banks; AV accumulator
[128,2,512] bufs1 = 2 banks; production/proj pool [128,2,512] bufs1 = 2 banks.
k/vt production is woven into unit 0's group boundaries instead of running as
a serial upfront block; AV emission runs LAG groups behind QK so unit
boundaries (normalize/proj on the dedicated acc banks) never stall the PE.
"""

import math
from collections import deque

import numpy as np

import concourse.bass as bass
import concourse.tile as tile
from concourse import bacc, mybir
from concourse import bass_utils

F32 = mybir.dt.float32
BF16 = mybir.dt.bfloat16
F16 = mybir.dt.float16

B, C, HH, WW = 4, 256, 64, 64
T = HH * WW            # 4096
NH = 4                 # heads per batch
CH = C // NH           # 64 channels per head
G = 32                 # groupnorm groups
CPG = C // G           # 8 channels per group
EPS = 1e-5
NCORES = 8
THALF = T // 2         # 2048  t-columns per core
SCALE = 1.0 / math.sqrt(math.sqrt(CH))

P_DT = BF16            # dtype of exp'd attention weights + vT
NSC = T // 128         # 32 s-chunks; group g == s-chunk g (both heads)
LAG = 5                # AV emission runs this many groups behind QK

_CACHE = {}
LAST_RESULTS = None


def _build_program():
    nc = bacc.Bacc("TRN2", target_bir_lowering=False, debug=False)

    d_xb = nc.dram_tensor("xb", [C, T], F16, kind="ExternalInput").ap()
    d_xqf = nc.dram_tensor("xqf", [C, THALF], F32, kind="ExternalInput").ap()
    d_wT = nc.dram_tensor("wT", [C, 3 * C], F32, kind="ExternalInput").ap()
    d_qkvb = nc.dram_tensor("qkvb", [3 * C, 1], F32, kind="ExternalInput").ap()
    d_vbrow = nc.dram_tensor("vbrow", [1, C], F32, kind="ExternalInput").ap()
    d_pjT = nc.dram_tensor("pjT", [C, C], F16, kind="ExternalInput").ap()
    d_pjb = nc.dram_tensor("pjb", [C, 1], F32, kind="ExternalInput").ap()
    d_gnw = nc.dram_tensor("gnw", [C, 1], F32, kind="ExternalInput").ap()
    d_gnb = nc.dram_tensor("gnb", [C, 1], F32, kind="ExternalInput").ap()
    d_gsel = nc.dram_tensor("gsel", [128, 2 * G], F32, kind="ExternalInput").ap()
    d_bsel = nc.dram_tensor("bsel", [G, C], F32, kind="ExternalInput").ap()
    d_out = nc.dram_tensor("out", [C, THALF], F32, kind="ExternalOutput").ap()

    with tile.TileContext(nc) as tc:
        with tc.tile_pool(name="data", bufs=1) as data, \
             tc.tile_pool(name="small", bufs=1) as small, \
             tc.tile_pool(name="work", bufs=3) as work:

            # ---- persistent SBUF tensors ----
            x = [data.tile([128, T], F16, tag=f"x{i}", name=f"x{i}") for i in range(2)]
            xqf = [data.tile([128, THALF], F32, tag=f"xqf{i}", name=f"xqf{i}") for i in range(2)]
            wt = [data.tile([128, 3 * C], F32, tag=f"wt{i}", name=f"wt{i}") for i in range(2)]
            wtf = [data.tile([128, 3 * C], F16, tag=f"wtf{i}", name=f"wtf{i}") for i in range(2)]
            pjt = [data.tile([128, C], F16, tag=f"pjt{i}", name=f"pjt{i}") for i in range(2)]
            q_sb = [data.tile([128, THALF], F16, tag=f"q{i}", name=f"q{i}") for i in range(2)]
            k_sb = [data.tile([128, T], F16, tag=f"k{i}", name=f"k{i}") for i in range(2)]
            # vT: per s-chunk, per head: 64 v-columns + 1 ones-column (+63 pad)
            vt = data.tile([128, NSC, NH, 2 * CH], P_DT, tag="vt", name="vt")
            a_sb = [data.tile([128, THALF], F16, tag=f"a{i}", name=f"a{i}") for i in range(2)]
            vbias_bc = data.tile([128, C], F32, tag="vbias_bc", name="vbias_bc")

            gnw = [small.tile([128, 1], F32, tag=f"gnw{i}", name=f"gnw{i}") for i in range(2)]
            gnb = [small.tile([128, 1], F32, tag=f"gnb{i}", name=f"gnb{i}") for i in range(2)]
            pjb = [small.tile([128, 1], F32, tag=f"pjb{i}", name=f"pjb{i}") for i in range(2)]
            qkb = [small.tile([128, 1], F32, tag=f"qkb{o}", name=f"qkb{o}") for o in range(4)]
            gsel = small.tile([128, 2 * G], F32, tag="gsel", name="gsel")
            bsel = small.tile([G, C], F32, tag="bsel", name="bsel")
            vbrow = small.tile([1, C], F32, tag="vbrow", name="vbrow")

            warm = small.tile([1, 1], F32, tag="warm", name="warm")
            nc.vector.memset(warm[:], 1.0)
            nc.scalar.activation(warm[:], warm[:], mybir.ActivationFunctionType.Ln)

            # ---- DMA: xb first (startup-critical), then weights, then the
            # rest. 1024-col chunks (2KB/partition lines).
            for ch4 in range(4):
                t4 = slice(1024 * ch4, 1024 * (ch4 + 1))
                for i in range(2):
                    cs = slice(128 * i, 128 * (i + 1))
                    nc.sync.dma_start(x[i][:, t4], d_xb[cs, t4])
            for i in range(2):
                cs = slice(128 * i, 128 * (i + 1))
                nc.sync.dma_start(wt[i][:], d_wT[cs, :])
            nc.sync.dma_start(gsel[:], d_gsel[:, :])
            nc.sync.dma_start(bsel[:], d_bsel[:, :])
            for i in range(2):
                cs = slice(128 * i, 128 * (i + 1))
                nc.sync.dma_start(gnw[i][:], d_gnw[cs, :])
                nc.sync.dma_start(gnb[i][:], d_gnb[cs, :])
            for o in range(4):
                nc.sync.dma_start(qkb[o][:], d_qkvb[128 * o:128 * (o + 1), :])
            nc.sync.dma_start(vbrow[:], d_vbrow[:, :])
            for i in range(2):
                cs = slice(128 * i, 128 * (i + 1))
                nc.sync.dma_start(pjb[i][:], d_pjb[cs, :])
                nc.sync.dma_start(pjt[i][:], d_pjT[cs, :])
            for i in range(2):
                cs = slice(128 * i, 128 * (i + 1))
                nc.sync.dma_start(xqf[i][:], d_xqf[cs, :])  # residual-only, late

            with tc.tile_pool(name="psQK", bufs=2, space="PSUM") as psQK, \
                 tc.tile_pool(name="psA", bufs=1, space="PSUM") as psA, \
                 tc.tile_pool(name="psP", bufs=1, space="PSUM") as psP:

                def pp_tile():
                    return psP.tile([128, 2, 512], F32, tag="pp", name="pp")

                # ============ GroupNorm stats -> per-channel A, B ==========
                A = [small.tile([128, 1], F32, tag=f"A{i}", name=f"A{i}") for i in range(2)]
                Bs = [small.tile([128, 1], F32, tag=f"B{i}", name=f"B{i}") for i in range(2)]
                pcs = [small.tile([128, 2], F32, tag=f"pcs{i}", name=f"pcs{i}") for i in range(2)]
                for i in range(2):
                    stats = work.tile([128, 8, 6], F32, tag="bnstats", name="bnstats")
                    for j in range(8):
                        nc.vector.bn_stats(stats[:, j, :], x[i][:, 512 * j:512 * (j + 1)])
                    mv = work.tile([128, 2], F32, tag="bnmv", name="bnmv")
                    nc.vector.bn_aggr(mv[:], stats[:])
                    # pcs = (mean, E[x^2]) per channel
                    nc.vector.tensor_copy(pcs[i][:, 0:1], mv[:, 0:1])
                    nc.vector.tensor_tensor(pcs[i][:, 1:2], mv[:, 0:1], mv[:, 0:1],
                                            mybir.AluOpType.mult)
                    nc.vector.tensor_tensor(pcs[i][:, 1:2], pcs[i][:, 1:2], mv[:, 1:2],
                                            mybir.AluOpType.add)

                # group stats [G, 2] = (mean_g, E[x^2]_g)
                grp_ps = pp_tile()[:G, 0, 0:2]
                nc.tensor.matmul(grp_ps, gsel[:, 0:G], pcs[0][:], start=True, stop=False)
                nc.tensor.matmul(grp_ps, gsel[:, G:2 * G], pcs[1][:], start=False, stop=True)

                grp_sb = small.tile([G, 2], F32, tag="grp_sb", name="grp_sb")
                nc.vector.tensor_copy(grp_sb[:], grp_ps)
                grp2 = small.tile([G, 2], F32, tag="grp2", name="grp2")  # (mean, rstd)
                var = small.tile([G, 1], F32, tag="var", name="var")
                epst = small.tile([G, 1], F32, tag="epst", name="epst")
                nc.vector.memset(epst[:], EPS)
                nc.vector.tensor_copy(grp2[:, 0:1], grp_sb[:, 0:1])
                nc.vector.tensor_tensor(var[:], grp_sb[:, 0:1], grp_sb[:, 0:1],
                                        mybir.AluOpType.mult)
                nc.vector.tensor_tensor(var[:], grp_sb[:, 1:2], var[:],
                                        mybir.AluOpType.subtract)
                nc.scalar.activation(var[:], var[:], mybir.ActivationFunctionType.Ln,
                                     bias=epst[:])
                nc.scalar.activation(grp2[:, 1:2], var[:],
                                     mybir.ActivationFunctionType.Exp, scale=-0.5)

                # broadcast to channels; A = rstd*gn_w, B = gn_b - mean*A
                for i in range(2):
                    mb_ps = pp_tile()[:, 0, 0:2]
                    nc.tensor.matmul(mb_ps, bsel[:, 128 * i:128 * (i + 1)], grp2[:],
                                     start=True, stop=True)
                    nc.vector.tensor_tensor(A[i][:], mb_ps[:, 1:2], gnw[i][:],
                                            mybir.AluOpType.mult)
                    nc.vector.tensor_tensor(Bs[i][:], mb_ps[:, 0:1], A[i][:],
                                            mybir.AluOpType.mult)
                    nc.vector.tensor_tensor(Bs[i][:], gnb[i][:], Bs[i][:],
                                            mybir.AluOpType.subtract)
                    # folded weights
                    nc.vector.tensor_scalar_mul(wtf[i][:], wt[i][:], A[i][:])

                # folded q/k biases: qkvb[o] + sum_c wT[c,o]*B[c]
                for o in range(4):
                    b_ps = pp_tile()[:, 0, 0:1]
                    nc.tensor.matmul(b_ps, wt[0][:, 128 * o:128 * (o + 1)], Bs[0][:],
                                     start=True, stop=False)
                    nc.tensor.matmul(b_ps, wt[1][:, 128 * o:128 * (o + 1)], Bs[1][:],
                                     start=False, stop=True)
                    nc.vector.tensor_tensor(qkb[o][:], qkb[o][:], b_ps,
                                            mybir.AluOpType.add)
                # folded v bias as a row [1, C], then broadcast to 128 partitions
                vb_ps = pp_tile()[0:1, 0, 0:C]
                nc.tensor.matmul(vb_ps, Bs[0][:], wt[0][:, 2 * C:3 * C],
                                 start=True, stop=False)
                nc.tensor.matmul(vb_ps, Bs[1][:], wt[1][:, 2 * C:3 * C],
                                 start=False, stop=True)
                vb_row = small.tile([1, C], F32, tag="vb_row", name="vb_row")
                nc.vector.tensor_tensor(vb_row[:], vb_ps, vbrow[:],
                                        mybir.AluOpType.add)
                nc.gpsimd.partition_broadcast(vbias_bc[:], vb_row[:])
                nc.gpsimd.memset(vt[:, :, :, CH:CH + 1], 1.0)  # ones col (denominator)
                nc.gpsimd.memset(vt[:, :, :, CH + 1:2 * CH], 0.0)  # zero pad

                # ================= production helpers =====================
                def qk_pair(dst, wofs, bias, o, t):
                    # two 512-col t-tiles of q or k output block o.  Queries
                    # read x cols 0..2047 directly (host pre-rotated).
                    ts2 = slice(512 * t, 512 * (t + 2))
                    ps = pp_tile()
                    for u in range(2):
                        ts = slice(512 * (t + u), 512 * (t + u + 1))
                        nc.tensor.matmul(ps[:, u, :],
                                         wtf[0][:, wofs + 128 * o:wofs + 128 * (o + 1)],
                                         x[0][:, ts], start=True, stop=False)
                        nc.tensor.matmul(ps[:, u, :],
                                         wtf[1][:, wofs + 128 * o:wofs + 128 * (o + 1)],
                                         x[1][:, ts], start=False, stop=True)
                    nc.vector.tensor_scalar(dst[o][:, ts2].rearrange("p (u f) -> p u f", u=2),
                                            ps[:], bias[:], SCALE,
                                            mybir.AluOpType.add, mybir.AluOpType.mult)

                def vt_quad(sc):
                    # four 128-wide s-chunks through one pp slot
                    ps = pp_tile().rearrange("p u (v f) -> p (u v) f", v=2)
                    for u in range(4):
                        ss = slice(128 * (sc + u), 128 * (sc + u + 1))
                        nc.tensor.matmul(ps[:, u, :], x[0][:, ss], wtf[0][:, 2 * C:3 * C],
                                         start=True, stop=False)
                        nc.tensor.matmul(ps[:, u, :], x[1][:, ss], wtf[1][:, 2 * C:3 * C],
                                         start=False, stop=True)
                    nc.vector.tensor_tensor(
                        vt[:, sc:sc + 4, :, 0:CH],
                        ps.rearrange("p u (h c) -> p u h c", h=NH),
                        bass.AP(tensor=vbias_bc[:].tensor, offset=vbias_bc[:].offset,
                                ap=[vbias_bc[:].ap[0], [0, 4], [CH, NH], [1, CH]]),
                        mybir.AluOpType.add)

                # minimal pre-attention production: q/k for pair 0's start +
                # the first vt quad.  Queries are always x cols 0..2047.
                qk_pair(q_sb, 0, qkb[0], 0, 0)
                qk_pair(k_sb, C, qkb[2], 0, 0)
                vt_quad(0)

                # deferred production, keyed by global group index
                prod_at = {}

                def at(g, fn):
                    prod_at.setdefault(g, []).append(fn)

                # vt quads 1..7 at groups 4j+1 (deadline: AV(4j) at 4j+LAG)
                for j in range(1, 8):
                    at(4 * j - 2, lambda j=j: vt_quad(4 * j))
                # k pair t covers s-chunks 4t..4t+7; QK needs it at group 4t
                at(5, lambda: qk_pair(k_sb, C, qkb[2], 0, 2))
                at(13, lambda: qk_pair(k_sb, C, qkb[2], 0, 4))
                at(21, lambda: qk_pair(k_sb, C, qkb[2], 0, 6))
                # remaining q/k for later units (unit u starts at group 32u)
                at(33, lambda: qk_pair(q_sb, 0, qkb[0], 0, 2))
                at(36, lambda: qk_pair(k_sb, C, qkb[3], 1, 0))
                at(40, lambda: qk_pair(k_sb, C, qkb[3], 1, 2))
                at(44, lambda: qk_pair(k_sb, C, qkb[3], 1, 4))
                at(48, lambda: qk_pair(k_sb, C, qkb[3], 1, 6))
                at(52, lambda: qk_pair(q_sb, 0, qkb[1], 1, 0))
                at(56, lambda: qk_pair(q_sb, 0, qkb[1], 1, 2))

                # ================= attention + proj =======================
                with tc.tile_pool(name="pexp", bufs=LAG + 2) as pexp, \
                     tc.tile_pool(name="nrm", bufs=4) as nrm, \
                     tc.tile_pool(name="outp", bufs=3) as outp:

                    def emit_av(acc, p, sc, pe):
                        for j in range(2):
                            h = 2 * p + j
                            nc.tensor.matmul(acc[:, j, :], vt[:, sc, h, :],
                                             pe[:, j, :],
                                             start=(sc == 0), stop=(sc == NSC - 1))

                    def normalize(p, tb, acc):
                        tbs = slice(512 * tb, 512 * (tb + 1))
                        for hi in range(2):
                            hp = slice(64 * hi, 64 * hi + 64)
                            den = nrm.tile([1, 512], F32, tag="den", name="den")
                            nc.vector.tensor_copy(den[:], acc[CH:CH + 1, hi, :])
                            rec = nrm.tile([1, 512], F32, tag="rec", name="rec")
                            nc.vector.reciprocal_approx_fast(rec[:], den[:])
                            bc = nrm.tile([CH, 512], F32, tag="bc", name="bc")
                            nc.gpsimd.partition_broadcast(bc[:], rec[:])
                            nc.vector.tensor_tensor(a_sb[p][hp, tbs], acc[0:CH, hi, :],
                                                    bc[:], mybir.AluOpType.mult)

                    def proj_tb(tb):
                        tbs = slice(512 * tb, 512 * (tb + 1))
                        pr = pp_tile()
                        for o in range(2):
                            nc.tensor.matmul(pr[:, o, :], pjt[0][:, 128 * o:128 * (o + 1)],
                                             a_sb[0][:, tbs], start=True, stop=False)
                            nc.tensor.matmul(pr[:, o, :], pjt[1][:, 128 * o:128 * (o + 1)],
                                             a_sb[1][:, tbs], start=False, stop=True)
                        for o in range(2):
                            res = outp.tile([128, 512], F32, tag="res", name="res")
                            nc.vector.tensor_scalar(res[:], pr[:, o, :], pjb[o][:], None,
                                                    mybir.AluOpType.add)
                            nc.vector.tensor_tensor(res[:], res[:],
                                                    xqf[o][:, tbs],
                                                    mybir.AluOpType.add)
                            nc.sync.dma_start(d_out[128 * o:128 * (o + 1), tbs], res[:])

                    pend = deque()   # (acc, p, sc, pe, post)
                    units = [(p, tb) for p in range(2) for tb in range(THALF // 512)]
                    gg = 0
                    for (p, tb) in units:
                        tbs = slice(512 * tb, 512 * (tb + 1))
                        acc = psA.tile([128, 2, 512], F32, tag="acc", name="acc")
                        for sc in range(NSC):
                            qkp = psQK.tile([128, 2, 512], F32, tag="qk", name="qk")
                            for j in range(2):
                                hp = slice(64 * j, 64 * j + 64)
                                nc.tensor.matmul(
                                    qkp[:, j, :],
                                    k_sb[p][hp, 128 * sc:128 * (sc + 1)],
                                    q_sb[p][hp, tbs],
                                    start=True, stop=True)
                            pe = pexp.tile([128, 2, 512], P_DT, tag="pe", name="pe")
                            nc.scalar.activation(pe[:], qkp[:],
                                                 mybir.ActivationFunctionType.Exp)
                            for fn in prod_at.pop(gg, []):
                                fn()
                            if len(pend) >= LAG:
                                ent = pend.popleft()
                                emit_av(*ent[:4])
                                if ent[4] is not None:
                                    ent[4]()
                            post = None
                            if sc == NSC - 1:
                                def post(p=p, tb=tb, acc=acc):
                                    normalize(p, tb, acc)
                                    if p == 1:
                                        proj_tb(tb)
                            pend.append((acc, p, sc, pe, post))
                            gg += 1
                    while pend:
                        ent = pend.popleft()
                        emit_av(*ent[:4])
                        if ent[4] is not None:
                            ent[4]()

    nc.compile()
    return nc


def _host_consts():
    g1 = np.zeros((128, G), dtype=np.float32)
    g2 = np.zeros((128, G), dtype=np.float32)
    for c in range(128):
        g1[c, c // CPG] = 1.0 / CPG
        g2[c, G // 2 + c // CPG] = 1.0 / CPG
    gsel = np.concatenate([g1, g2], axis=1)          # [128, 2G]
    bsel = np.zeros((G, C), dtype=np.float32)
    for c in range(C):
        bsel[c // CPG, c] = 1.0
    return gsel, bsel


def kernel(x, gn_w, gn_b, qkv_w, qkv_b, proj_w, proj_b):
    global LAST_RESULTS
    if "nc" not in _CACHE:
        _CACHE["nc"] = _build_program()
    nc = _CACHE["nc"]

    x = np.ascontiguousarray(np.asarray(x, dtype=np.float32))
    xr = x.reshape(B, C, T)
    gsel, bsel = _host_consts()
    shared = {
        "wT": np.ascontiguousarray(np.asarray(qkv_w, np.float32).T),
        "qkvb": np.asarray(qkv_b, np.float32).reshape(3 * C, 1).copy(),
        "vbrow": np.asarray(qkv_b, np.float32)[2 * C:].reshape(1, C).copy(),
        "pjT": np.ascontiguousarray(np.asarray(proj_w, np.float32).T.astype(np.float16)),
        "pjb": np.asarray(proj_b, np.float32).reshape(C, 1).copy(),
        "gnw": np.asarray(gn_w, np.float32).reshape(C, 1).copy(),
        "gnb": np.asarray(gn_b, np.float32).reshape(C, 1).copy(),
        "gsel": gsel,
        "bsel": bsel,
    }
    in_maps = []
    for c in range(NCORES):
        b, hf = c // 2, c % 2
        m = dict(shared)
        # rotate so this core's query half is always columns 0..2047
        xrot = np.roll(xr[b], -hf * THALF, axis=1)
        m["xb"] = np.ascontiguousarray(xrot.astype(np.float16))
        m["xqf"] = np.ascontiguousarray(xrot[:, :THALF])
        in_maps.append(m)

    res = bass_utils.run_bass_kernel_spmd(nc, in_maps, core_ids=list(range(NCORES)))
    LAST_RESULTS = res

    out = np.empty((B, C, T), dtype=np.float32)
    for c in range(NCORES):
        b, hf = c // 2, c % 2
        out[b][:, hf * THALF:(hf + 1) * THALF] = res.results[c]["out"]
    return out.reshape(B, C, HH, WW)


# revision 14
# speedup vs baseline: 1.0459x; 1.0426x over previous
"""AttentionBlock (GroupNorm + 1x1 QKV + MHA + proj + residual) on 8 trn2 cores.

Sharding: core c -> (batch b = c//2, t-half = c%2). Each core computes all 4
heads for its 2048 query positions; k/v are computed over the full T=4096 from
the core's batch. No cross-core communication needed.

The host ROTATES each core's [C, T] input so the core's query half is always
columns 0..2047; k/v see the rotated full T (softmax over s is order
invariant as long as k_sb and vt share the order).

Layout trick: attention scores are computed TRANSPOSED, S_T[s, t] (s on
partitions), so the AV matmul needs no transposes: a[ch, t] = vT[s, ch].T @
P_T[s, t]. The softmax denominator comes from a ones-column appended to vT.
GroupNorm is folded into the QKV weights (per-channel scale/shift).

PSUM (8 banks): qk groups-of-2 [128,2,512] x bufs2 = 4 banks; AV accumulator
[128,2,512] bufs1 = 2 banks; production/proj pool [128,2,512] bufs1 = 2 banks.
k/vt production is woven into unit 0's group boundaries instead of running as
a serial upfront block; AV emission runs LAG groups behind QK so unit
boundaries (normalize/proj on the dedicated acc banks) never stall the PE.
"""

import math
from collections import deque

import numpy as np

import concourse.bass as bass
import concourse.tile as tile
from concourse import bacc, mybir
from concourse import bass_utils

F32 = mybir.dt.float32
BF16 = mybir.dt.bfloat16
F16 = mybir.dt.float16

B, C, HH, WW = 4, 256, 64, 64
T = HH * WW            # 4096
NH = 4                 # heads per batch
CH = C // NH           # 64 channels per head
G = 32                 # groupnorm groups
CPG = C // G           # 8 channels per group
EPS = 1e-5
NCORES = 8
THALF = T // 2         # 2048  t-columns per core
SCALE = 1.0 / math.sqrt(math.sqrt(CH))

P_DT = BF16            # dtype of exp'd attention weights + vT
NSC = T // 128         # 32 s-chunks; group g == s-chunk g (both heads)
LAG = 5                # AV emission runs this many groups behind QK

_CACHE = {}
LAST_RESULTS = None


def _build_program():
    nc = bacc.Bacc("TRN2", target_bir_lowering=False, debug=False)

    d_xb = nc.dram_tensor("xb", [C, T], F16, kind="ExternalInput").ap()
    d_xqf = nc.dram_tensor("xqf", [C, THALF], F32, kind="ExternalInput").ap()
    d_wT = nc.dram_tensor("wT", [C, 3 * C], F32, kind="ExternalInput").ap()
    d_qkvb = nc.dram_tensor("qkvb", [3 * C, 1], F32, kind="ExternalInput").ap()
    d_vbrow = nc.dram_tensor("vbrow", [1, C], F32, kind="ExternalInput").ap()
    d_pjT = nc.dram_tensor("pjT", [C, C], F16, kind="ExternalInput").ap()
    d_pjb = nc.dram_tensor("pjb", [C, 1], F32, kind="ExternalInput").ap()
    d_gnw = nc.dram_tensor("gnw", [C, 1], F32, kind="ExternalInput").ap()
    d_gnb = nc.dram_tensor("gnb", [C, 1], F32, kind="ExternalInput").ap()
    d_gsel = nc.dram_tensor("gsel", [128, 2 * G], F32, kind="ExternalInput").ap()
    d_bsel = nc.dram_tensor("bsel", [G, C], F32, kind="ExternalInput").ap()
    d_out = nc.dram_tensor("out", [C, THALF], F32, kind="ExternalOutput").ap()

    with tile.TileContext(nc) as tc:
        with tc.tile_pool(name="data", bufs=1) as data, \
             tc.tile_pool(name="small", bufs=1) as small, \
             tc.tile_pool(name="work", bufs=3) as work:

            # ---- persistent SBUF tensors ----
            x = [data.tile([128, T], F16, tag=f"x{i}", name=f"x{i}") for i in range(2)]
            xqf = [data.tile([128, THALF], F32, tag=f"xqf{i}", name=f"xqf{i}") for i in range(2)]
            wt = [data.tile([128, 3 * C], F32, tag=f"wt{i}", name=f"wt{i}") for i in range(2)]
            wtf = [data.tile([128, 3 * C], F16, tag=f"wtf{i}", name=f"wtf{i}") for i in range(2)]
            pjt = [data.tile([128, C], F16, tag=f"pjt{i}", name=f"pjt{i}") for i in range(2)]
            q_sb = [data.tile([128, THALF], F16, tag=f"q{i}", name=f"q{i}") for i in range(2)]
            k_sb = [data.tile([128, T], F16, tag=f"k{i}", name=f"k{i}") for i in range(2)]
            # vT: per s-chunk, per head: 64 v-columns + 1 ones-column (+63 pad)
            vt = data.tile([128, NSC, NH, 2 * CH], P_DT, tag="vt", name="vt")
            a_sb = [data.tile([128, THALF], F16, tag=f"a{i}", name=f"a{i}") for i in range(2)]
            vbias_bc = data.tile([128, C], F32, tag="vbias_bc", name="vbias_bc")

            gnw = [small.tile([128, 1], F32, tag=f"gnw{i}", name=f"gnw{i}") for i in range(2)]
            gnb = [small.tile([128, 1], F32, tag=f"gnb{i}", name=f"gnb{i}") for i in range(2)]
            pjb = [small.tile([128, 1], F32, tag=f"pjb{i}", name=f"pjb{i}") for i in range(2)]
            qkb = [small.tile([128, 1], F32, tag=f"qkb{o}", name=f"qkb{o}") for o in range(4)]
            gsel = small.tile([128, 2 * G], F32, tag="gsel", name="gsel")
            bsel = small.tile([G, C], F32, tag="bsel", name="bsel")
            vbrow = small.tile([1, C], F32, tag="vbrow", name="vbrow")

            warm = small.tile([1, 1], F32, tag="warm", name="warm")
            nc.vector.memset(warm[:], 1.0)
            nc.scalar.activation(warm[:], warm[:], mybir.ActivationFunctionType.Ln)
            nc.scalar.activation(warm[:], warm[:], mybir.ActivationFunctionType.Exp)

            # ---- DMA: xb first (startup-critical), then weights, then the
            # rest. 1024-col chunks (2KB/partition lines).
            for ch4 in range(4):
                t4 = slice(1024 * ch4, 1024 * (ch4 + 1))
                for i in range(2):
                    cs = slice(128 * i, 128 * (i + 1))
                    nc.sync.dma_start(x[i][:, t4], d_xb[cs, t4])
            for i in range(2):
                cs = slice(128 * i, 128 * (i + 1))
                nc.sync.dma_start(wt[i][:], d_wT[cs, :])
            nc.sync.dma_start(gsel[:], d_gsel[:, :])
            nc.sync.dma_start(bsel[:], d_bsel[:, :])
            for i in range(2):
                cs = slice(128 * i, 128 * (i + 1))
                nc.sync.dma_start(gnw[i][:], d_gnw[cs, :])
                nc.sync.dma_start(gnb[i][:], d_gnb[cs, :])
            for o in range(4):
                nc.sync.dma_start(qkb[o][:], d_qkvb[128 * o:128 * (o + 1), :])
            nc.sync.dma_start(vbrow[:], d_vbrow[:, :])
            for i in range(2):
                cs = slice(128 * i, 128 * (i + 1))
                nc.sync.dma_start(pjb[i][:], d_pjb[cs, :])
                nc.sync.dma_start(pjt[i][:], d_pjT[cs, :])
            for i in range(2):
                cs = slice(128 * i, 128 * (i + 1))
                nc.sync.dma_start(xqf[i][:], d_xqf[cs, :])  # residual-only, late

            with tc.tile_pool(name="psQK", bufs=2, space="PSUM") as psQK, \
                 tc.tile_pool(name="psA", bufs=1, space="PSUM") as psA, \
                 tc.tile_pool(name="psP", bufs=1, space="PSUM") as psP:

                def pp_tile():
                    return psP.tile([128, 2, 512], F32, tag="pp", name="pp")

                def qk_ps():
                    return psQK.tile([128, 2, 512], F32, tag="qk", name="qk")

                # ============ GroupNorm stats -> per-channel A, B ==========
                # Small fold matmuls rotate through the two psQK slots; their
                # readers are tiny DVE ops so rotation waits are negligible.
                A = [small.tile([128, 1], F32, tag=f"A{i}", name=f"A{i}") for i in range(2)]
                Bs = [small.tile([128, 1], F32, tag=f"B{i}", name=f"B{i}") for i in range(2)]
                pcs = [small.tile([128, 2], F32, tag=f"pcs{i}", name=f"pcs{i}") for i in range(2)]
                for i in range(2):
                    stats = work.tile([128, 8, 6], F32, tag="bnstats", name="bnstats")
                    for j in range(8):
                        nc.vector.bn_stats(stats[:, j, :], x[i][:, 512 * j:512 * (j + 1)])
                    mv = work.tile([128, 2], F32, tag="bnmv", name="bnmv")
                    nc.vector.bn_aggr(mv[:], stats[:])
                    # pcs = (mean, E[x^2]) per channel
                    nc.vector.tensor_copy(pcs[i][:, 0:1], mv[:, 0:1])
                    nc.vector.tensor_tensor(pcs[i][:, 1:2], mv[:, 0:1], mv[:, 0:1],
                                            mybir.AluOpType.mult)
                    nc.vector.tensor_tensor(pcs[i][:, 1:2], pcs[i][:, 1:2], mv[:, 1:2],
                                            mybir.AluOpType.add)

                # group stats [G, 2] = (mean_g, E[x^2]_g)
                grp_ps = qk_ps()[:G, 0, 0:2]
                nc.tensor.matmul(grp_ps, gsel[:, 0:G], pcs[0][:], start=True, stop=False)
                nc.tensor.matmul(grp_ps, gsel[:, G:2 * G], pcs[1][:], start=False, stop=True)

                grp_sb = small.tile([G, 2], F32, tag="grp_sb", name="grp_sb")
                nc.vector.tensor_copy(grp_sb[:], grp_ps)
                grp2 = small.tile([G, 2], F32, tag="grp2", name="grp2")  # (mean, rstd)
                var = small.tile([G, 1], F32, tag="var", name="var")
                epst = small.tile([G, 1], F32, tag="epst", name="epst")
                nc.vector.memset(epst[:], EPS)
                nc.vector.tensor_copy(grp2[:, 0:1], grp_sb[:, 0:1])
                nc.vector.tensor_tensor(var[:], grp_sb[:, 0:1], grp_sb[:, 0:1],
                                        mybir.AluOpType.mult)
                nc.vector.tensor_tensor(var[:], grp_sb[:, 1:2], var[:],
                                        mybir.AluOpType.subtract)
                nc.scalar.activation(var[:], var[:], mybir.ActivationFunctionType.Ln,
                                     bias=epst[:])
                nc.scalar.activation(grp2[:, 1:2], var[:],
                                     mybir.ActivationFunctionType.Exp, scale=-0.5)

                # broadcast to channels; A = rstd*gn_w, B = gn_b - mean*A
                for i in range(2):
                    mb_ps = qk_ps()[:, 0, 0:2]
                    nc.tensor.matmul(mb_ps, bsel[:, 128 * i:128 * (i + 1)], grp2[:],
                                     start=True, stop=True)
                    nc.vector.tensor_tensor(A[i][:], mb_ps[:, 1:2], gnw[i][:],
                                            mybir.AluOpType.mult)
                    nc.vector.tensor_tensor(Bs[i][:], mb_ps[:, 0:1], A[i][:],
                                            mybir.AluOpType.mult)
                    nc.vector.tensor_tensor(Bs[i][:], gnb[i][:], Bs[i][:],
                                            mybir.AluOpType.subtract)
                    # folded weights
                    nc.vector.tensor_scalar_mul(wtf[i][:], wt[i][:], A[i][:])

                # folded q/k biases: qkvb[o] + sum_c wT[c,o]*B[c]
                for o in range(4):
                    b_ps = qk_ps()[:, 0, 0:1]
                    nc.tensor.matmul(b_ps, wt[0][:, 128 * o:128 * (o + 1)], Bs[0][:],
                                     start=True, stop=False)
                    nc.tensor.matmul(b_ps, wt[1][:, 128 * o:128 * (o + 1)], Bs[1][:],
                                     start=False, stop=True)
                    nc.vector.tensor_tensor(qkb[o][:], qkb[o][:], b_ps,
                                            mybir.AluOpType.add)
                # folded v bias as a row [1, C], then broadcast to 128 partitions
                vb_ps = qk_ps()[0:1, 0, 0:C]
                nc.tensor.matmul(vb_ps, Bs[0][:], wt[0][:, 2 * C:3 * C],
                                 start=True, stop=False)
                nc.tensor.matmul(vb_ps, Bs[1][:], wt[1][:, 2 * C:3 * C],
                                 start=False, stop=True)
                vb_row = small.tile([1, C], F32, tag="vb_row", name="vb_row")
                nc.vector.tensor_tensor(vb_row[:], vb_ps, vbrow[:],
                                        mybir.AluOpType.add)
                nc.gpsimd.partition_broadcast(vbias_bc[:], vb_row[:])
                nc.gpsimd.memset(vt[:, :, :, CH:CH + 1], 1.0)  # ones col (denominator)
                nc.gpsimd.memset(vt[:, :, :, CH + 1:2 * CH], 0.0)  # zero pad

                # ================= production helpers =====================
                def qk_pair(dst, wofs, bias, o, t, ps=None):
                    # two 512-col t-tiles of q or k output block o.  Queries
                    # read x cols 0..2047 directly (host pre-rotated).
                    ts2 = slice(512 * t, 512 * (t + 2))
                    if ps is None:
                        ps = pp_tile()
                    for u in range(2):
                        ts = slice(512 * (t + u), 512 * (t + u + 1))
                        nc.tensor.matmul(ps[:, u, :],
                                         wtf[0][:, wofs + 128 * o:wofs + 128 * (o + 1)],
                                         x[0][:, ts], start=True, stop=False)
                        nc.tensor.matmul(ps[:, u, :],
                                         wtf[1][:, wofs + 128 * o:wofs + 128 * (o + 1)],
                                         x[1][:, ts], start=False, stop=True)
                    nc.vector.tensor_scalar(dst[o][:, ts2].rearrange("p (u f) -> p u f", u=2),
                                            ps[:], bias[:], SCALE,
                                            mybir.AluOpType.add, mybir.AluOpType.mult)

                def vt_quad(sc, ps=None):
                    # four 128-wide s-chunks through one pp slot
                    ps = (pp_tile() if ps is None else ps).rearrange(
                        "p u (v f) -> p (u v) f", v=2)
                    for u in range(4):
                        ss = slice(128 * (sc + u), 128 * (sc + u + 1))
                        nc.tensor.matmul(ps[:, u, :], x[0][:, ss], wtf[0][:, 2 * C:3 * C],
                                         start=True, stop=False)
                        nc.tensor.matmul(ps[:, u, :], x[1][:, ss], wtf[1][:, 2 * C:3 * C],
                                         start=False, stop=True)
                    nc.vector.tensor_tensor(
                        vt[:, sc:sc + 4, :, 0:CH],
                        ps.rearrange("p u (h c) -> p u h c", h=NH),
                        bass.AP(tensor=vbias_bc[:].tensor, offset=vbias_bc[:].offset,
                                ap=[vbias_bc[:].ap[0], [0, 4], [CH, NH], [1, CH]]),
                        mybir.AluOpType.add)

                # minimal pre-attention production: q/k for pair 0's start +
                # the first vt quad, spread over three psum slots so the
                # matmuls run back-to-back instead of serializing on one pool.
                qk_pair(q_sb, 0, qkb[0], 0, 0)
                qk_pair(k_sb, C, qkb[2], 0, 0, ps=qk_ps())
                vt_quad(0, ps=qk_ps())

                # deferred production, keyed by global group index
                prod_at = {}

                def at(g, fn):
                    prod_at.setdefault(g, []).append(fn)

                # vt quads 1..7 at groups 4j+1 (deadline: AV(4j) at 4j+LAG)
                for j in range(1, 8):
                    at(4 * j - 2, lambda j=j: vt_quad(4 * j))
                # k pair t covers s-chunks 4t..4t+7; QK needs it at group 4t
                at(5, lambda: qk_pair(k_sb, C, qkb[2], 0, 2))
                at(13, lambda: qk_pair(k_sb, C, qkb[2], 0, 4))
                at(21, lambda: qk_pair(k_sb, C, qkb[2], 0, 6))
                # remaining q/k for later units (unit u starts at group 32u)
                at(33, lambda: qk_pair(q_sb, 0, qkb[0], 0, 2))
                at(36, lambda: qk_pair(k_sb, C, qkb[3], 1, 0))
                at(40, lambda: qk_pair(k_sb, C, qkb[3], 1, 2))
                at(44, lambda: qk_pair(k_sb, C, qkb[3], 1, 4))
                at(48, lambda: qk_pair(k_sb, C, qkb[3], 1, 6))
                at(52, lambda: qk_pair(q_sb, 0, qkb[1], 1, 0))
                at(56, lambda: qk_pair(q_sb, 0, qkb[1], 1, 2))

                # ================= attention + proj =======================
                with tc.tile_pool(name="pexp", bufs=LAG + 2) as pexp, \
                     tc.tile_pool(name="nrm", bufs=4) as nrm, \
                     tc.tile_pool(name="outp", bufs=3) as outp:

                    def emit_av(acc, p, sc, pe):
                        for j in range(2):
                            h = 2 * p + j
                            nc.tensor.matmul(acc[:, j, :], vt[:, sc, h, :],
                                             pe[:, j, :],
                                             start=(sc == 0), stop=(sc == NSC - 1))

                    def normalize(p, tb, acc):
                        tbs = slice(512 * tb, 512 * (tb + 1))
                        for hi in range(2):
                            hp = slice(64 * hi, 64 * hi + 64)
                            den = nrm.tile([1, 512], F32, tag="den", name="den")
                            nc.vector.tensor_copy(den[:], acc[CH:CH + 1, hi, :])
                            rec = nrm.tile([1, 512], F32, tag="rec", name="rec")
                            nc.vector.reciprocal_approx_fast(rec[:], den[:])
                            bc = nrm.tile([CH, 512], F32, tag="bc", name="bc")
                            nc.gpsimd.partition_broadcast(bc[:], rec[:])
                            nc.vector.tensor_tensor(a_sb[p][hp, tbs], acc[0:CH, hi, :],
                                                    bc[:], mybir.AluOpType.mult)

                    def proj_tb(tb):
                        tbs = slice(512 * tb, 512 * (tb + 1))
                        pr = pp_tile()
                        for o in range(2):
                            nc.tensor.matmul(pr[:, o, :], pjt[0][:, 128 * o:128 * (o + 1)],
                                             a_sb[0][:, tbs], start=True, stop=False)
                            nc.tensor.matmul(pr[:, o, :], pjt[1][:, 128 * o:128 * (o + 1)],
                                             a_sb[1][:, tbs], start=False, stop=True)
                        for o in range(2):
                            res = outp.tile([128, 512], F32, tag="res", name="res")
                            nc.vector.tensor_scalar(res[:], pr[:, o, :], pjb[o][:], None,
                                                    mybir.AluOpType.add)
                            nc.vector.tensor_tensor(res[:], res[:],
                                                    xqf[o][:, tbs],
                                                    mybir.AluOpType.add)
                            nc.sync.dma_start(d_out[128 * o:128 * (o + 1), tbs], res[:])

                    pend = deque()   # (acc, p, sc, pe, post)
                    units = [(p, tb) for p in range(2) for tb in range(THALF // 512)]
                    gg = 0
                    for (p, tb) in units:
                        tbs = slice(512 * tb, 512 * (tb + 1))
                        acc = psA.tile([128, 2, 512], F32, tag="acc", name="acc")
                        for sc in range(NSC):
                            qkp = psQK.tile([128, 2, 512], F32, tag="qk", name="qk")
                            for j in range(2):
                                hp = slice(64 * j, 64 * j + 64)
                                nc.tensor.matmul(
                                    qkp[:, j, :],
                                    k_sb[p][hp, 128 * sc:128 * (sc + 1)],
                                    q_sb[p][hp, tbs],
                                    start=True, stop=True)
                            pe = pexp.tile([128, 2, 512], P_DT, tag="pe", name="pe")
                            nc.scalar.activation(pe[:], qkp[:],
                                                 mybir.ActivationFunctionType.Exp)
                            for fn in prod_at.pop(gg, []):
                                fn()
                            post = None
                            if sc == NSC - 1:
                                def post(p=p, tb=tb, acc=acc):
                                    normalize(p, tb, acc)
                                    if p == 1:
                                        proj_tb(tb)
                            pend.append((acc, p, sc, pe, post))
                            # taper-drain: pop 2 per group near the unit end so
                            # the lag is zero at the boundary and normalize can
                            # free the acc banks before the next unit's AVs.
                            npop = 2 if sc >= NSC - LAG else (
                                1 if len(pend) > LAG else 0)
                            for _ in range(npop):
                                if not pend:
                                    break
                                ent = pend.popleft()
                                emit_av(*ent[:4])
                                if ent[4] is not None:
                                    ent[4]()
                            gg += 1
                    while pend:
                        ent = pend.popleft()
                        emit_av(*ent[:4])
                        if ent[4] is not None:
                            ent[4]()

    nc.compile()
    return nc


def _host_consts():
    g1 = np.zeros((128, G), dtype=np.float32)
    g2 = np.zeros((128, G), dtype=np.float32)
    for c in range(128):
        g1[c, c // CPG] = 1.0 / CPG
        g2[c, G // 2 + c // CPG] = 1.0 / CPG
    gsel = np.concatenate([g1, g2], axis=1)          # [128, 2G]
    bsel = np.zeros((G, C), dtype=np.float32)
    for c in range(C):
        bsel[c // CPG, c] = 1.0
    return gsel, bsel


def kernel(x, gn_w, gn_b, qkv_w, qkv_b, proj_w, proj_b):
    global LAST_RESULTS
    if "nc" not in _CACHE:
        _CACHE["nc"] = _build_program()
    nc = _CACHE["nc"]

    x = np.ascontiguousarray(np.asarray(x, dtype=np.float32))
    xr = x.reshape(B, C, T)
    gsel, bsel = _host_consts()
    shared = {
        "wT": np.ascontiguousarray(np.asarray(qkv_w, np.float32).T),
        "qkvb": np.asarray(qkv_b, np.float32).reshape(3 * C, 1).copy(),
        "vbrow": np.asarray(qkv_b, np.float32)[2 * C:].reshape(1, C).copy(),
        "pjT": np.ascontiguousarray(np.asarray(proj_w, np.float32).T.astype(np.float16)),
        "pjb": np.asarray(proj_b, np.float32).reshape(C, 1).copy(),
        "gnw": np.asarray(gn_w, np.float32).reshape(C, 1).copy(),
        "gnb": np.asarray(gn_b, np.float32).reshape(C, 1).copy(),
        "gsel": gsel,
        "bsel": bsel,
    }
    in_maps = []
    for c in range(NCORES):
        b, hf = c // 2, c % 2
        m = dict(shared)
        # rotate so this core's query half is always columns 0..2047
        xrot = np.roll(xr[b], -hf * THALF, axis=1)
        m["xb"] = np.ascontiguousarray(xrot.astype(np.float16))
        m["xqf"] = np.ascontiguousarray(xrot[:, :THALF])
        in_maps.append(m)

    res = bass_utils.run_bass_kernel_spmd(nc, in_maps, core_ids=list(range(NCORES)))
    LAST_RESULTS = res

    out = np.empty((B, C, T), dtype=np.float32)
    for c in range(NCORES):
        b, hf = c // 2, c % 2
        out[b][:, hf * THALF:(hf + 1) * THALF] = res.results[c]["out"]
    return out.reshape(B, C, HH, WW)


# revision 23
# speedup vs baseline: 1.0830x; 1.0355x over previous
"""AttentionBlock (GroupNorm + 1x1 QKV + MHA + proj + residual) on 8 trn2 cores.

Sharding: core c -> (batch b = c//2, t-half = c%2). Each core computes all 4
heads for its 2048 query positions; k/v are computed over the full T=4096 from
the core's batch. No cross-core communication needed.

The host ROTATES each core's [C, T] input so the core's query half is always
columns 0..2047; k/v see the rotated full T (softmax over s is order
invariant as long as k_sb and vt share the order).

Layout trick: attention scores are computed TRANSPOSED, S_T[s, t] (s on
partitions), so the AV matmul needs no transposes: a[ch, t] = vT[s, ch].T @
P_T[s, t]. The softmax denominator comes from a ones-column appended to vT.
GroupNorm is folded into the QKV weights (per-channel scale/shift).

PSUM (8 banks): qk groups-of-2 [128,2,512] x bufs2 = 4 banks; AV accumulator
[128,2,512] bufs1 = 2 banks; production/proj pool [128,2,512] bufs1 = 2 banks.
k/vt production is woven into unit 0's group boundaries instead of running as
a serial upfront block; AV emission runs LAG groups behind QK so unit
boundaries (normalize/proj on the dedicated acc banks) never stall the PE.
"""

import math
from collections import deque

import numpy as np

import concourse.bass as bass
import concourse.tile as tile
from concourse import bacc, mybir
from concourse import bass_utils

F32 = mybir.dt.float32
BF16 = mybir.dt.bfloat16
F16 = mybir.dt.float16

B, C, HH, WW = 4, 256, 64, 64
T = HH * WW            # 4096
NH = 4                 # heads per batch
CH = C // NH           # 64 channels per head
G = 32                 # groupnorm groups
CPG = C // G           # 8 channels per group
EPS = 1e-5
NCORES = 8
THALF = T // 2         # 2048  t-columns per core
SCALE = 1.0 / math.sqrt(math.sqrt(CH))

P_DT = BF16            # dtype of exp'd attention weights + vT
NSC = T // 128         # 32 s-chunks; group g == s-chunk g (both heads)
LAG = 5                # AV emission runs this many groups behind QK

_CACHE = {}
LAST_RESULTS = None


def _build_program():
    nc = bacc.Bacc("TRN2", target_bir_lowering=False, debug=False)

    d_xb = nc.dram_tensor("xb", [C, T], F16, kind="ExternalInput").ap()
    d_xqf = nc.dram_tensor("xqf", [C, THALF], F32, kind="ExternalInput").ap()
    d_wT = nc.dram_tensor("wT", [C, 3 * C], F32, kind="ExternalInput").ap()
    d_qkvb = nc.dram_tensor("qkvb", [3 * C, 1], F32, kind="ExternalInput").ap()
    d_vbrow = nc.dram_tensor("vbrow", [1, C], F32, kind="ExternalInput").ap()
    d_pjT = nc.dram_tensor("pjT", [C, C], F16, kind="ExternalInput").ap()
    d_pjb = nc.dram_tensor("pjb", [C, 1], F32, kind="ExternalInput").ap()
    d_gnw = nc.dram_tensor("gnw", [C, 1], F32, kind="ExternalInput").ap()
    d_gnb = nc.dram_tensor("gnb", [C, 1], F32, kind="ExternalInput").ap()
    d_gsel = nc.dram_tensor("gsel", [128, 2 * G], F32, kind="ExternalInput").ap()
    d_bsel = nc.dram_tensor("bsel", [G, C], F32, kind="ExternalInput").ap()
    d_out = nc.dram_tensor("out", [C, THALF], F32, kind="ExternalOutput").ap()

    with tile.TileContext(nc) as tc:
        with tc.tile_pool(name="data", bufs=1) as data, \
             tc.tile_pool(name="small", bufs=1) as small, \
             tc.tile_pool(name="work", bufs=3) as work:

            # ---- persistent SBUF tensors ----
            x = [data.tile([128, T], F16, tag=f"x{i}", name=f"x{i}") for i in range(2)]
            xqf = [data.tile([128, THALF], F32, tag=f"xqf{i}", name=f"xqf{i}") for i in range(2)]
            wt = [data.tile([128, 3 * C], F32, tag=f"wt{i}", name=f"wt{i}") for i in range(2)]
            wtf = [data.tile([128, 3 * C], F16, tag=f"wtf{i}", name=f"wtf{i}") for i in range(2)]
            pjt = [data.tile([128, C], F16, tag=f"pjt{i}", name=f"pjt{i}") for i in range(2)]
            q_sb = [data.tile([128, THALF], F16, tag=f"q{i}", name=f"q{i}") for i in range(2)]
            k_sb = [data.tile([128, T], F16, tag=f"k{i}", name=f"k{i}") for i in range(2)]
            # vT: per s-chunk, per head: 64 v-columns + 1 ones-column (+63 pad)
            vt = data.tile([128, NSC, NH, 2 * CH], P_DT, tag="vt", name="vt")
            a_sb = [data.tile([128, THALF], F16, tag=f"a{i}", name=f"a{i}") for i in range(2)]
            vbias_bc = data.tile([128, C], F32, tag="vbias_bc", name="vbias_bc")

            gnw = [small.tile([128, 1], F32, tag=f"gnw{i}", name=f"gnw{i}") for i in range(2)]
            gnb = [small.tile([128, 1], F32, tag=f"gnb{i}", name=f"gnb{i}") for i in range(2)]
            pjb = [small.tile([128, 1], F32, tag=f"pjb{i}", name=f"pjb{i}") for i in range(2)]
            qkb = [small.tile([128, 1], F32, tag=f"qkb{o}", name=f"qkb{o}") for o in range(4)]
            gsel = small.tile([128, 2 * G], F32, tag="gsel", name="gsel")
            bsel = small.tile([G, C], F32, tag="bsel", name="bsel")
            vbrow = small.tile([1, C], F32, tag="vbrow", name="vbrow")

            # Exp is the only table-based activation in the whole kernel
            # (groupnorm rstd is computed on DVE); warm its table once here.
            warm = small.tile([1, 1], F32, tag="warm", name="warm")
            nc.vector.memset(warm[:], 1.0)
            nc.scalar.activation(warm[:], warm[:], mybir.ActivationFunctionType.Exp)

            # ---- DMA: xb first (startup-critical), then weights, then the
            # rest. 512-col chunks so all 16 DMA engines work in parallel
            # (one descriptor lands on one ~20GB/s engine).
            for ch8 in range(8):
                t8 = slice(512 * ch8, 512 * (ch8 + 1))
                for i in range(2):
                    cs = slice(128 * i, 128 * (i + 1))
                    nc.sync.dma_start(x[i][:, t8], d_xb[cs, t8])
            for i in range(2):
                cs = slice(128 * i, 128 * (i + 1))
                nc.sync.dma_start(wt[i][:], d_wT[cs, :])
            nc.sync.dma_start(gsel[:], d_gsel[:, :])
            nc.sync.dma_start(bsel[:], d_bsel[:, :])
            for i in range(2):
                cs = slice(128 * i, 128 * (i + 1))
                nc.sync.dma_start(gnw[i][:], d_gnw[cs, :])
                nc.sync.dma_start(gnb[i][:], d_gnb[cs, :])
            for o in range(4):
                nc.sync.dma_start(qkb[o][:], d_qkvb[128 * o:128 * (o + 1), :])
            nc.sync.dma_start(vbrow[:], d_vbrow[:, :])
            for i in range(2):
                cs = slice(128 * i, 128 * (i + 1))
                nc.sync.dma_start(pjb[i][:], d_pjb[cs, :])
                nc.sync.dma_start(pjt[i][:], d_pjT[cs, :])
            for i in range(2):
                cs = slice(128 * i, 128 * (i + 1))
                nc.sync.dma_start(xqf[i][:], d_xqf[cs, :])  # residual-only, late

            with tc.tile_pool(name="psQK", bufs=2, space="PSUM") as psQK, \
                 tc.tile_pool(name="psA", bufs=1, space="PSUM") as psA, \
                 tc.tile_pool(name="psP", bufs=1, space="PSUM") as psP:

                def pp_tile():
                    return psP.tile([128, 2, 512], F32, tag="pp", name="pp")

                def qk_ps():
                    return psQK.tile([128, 2, 512], F32, tag="qk", name="qk")

                # ============ GroupNorm stats -> per-channel A, B ==========
                # Small fold matmuls rotate through the two psQK slots; their
                # readers are tiny DVE ops so rotation waits are negligible.
                A = [small.tile([128, 1], F32, tag=f"A{i}", name=f"A{i}") for i in range(2)]
                Bs = [small.tile([128, 1], F32, tag=f"B{i}", name=f"B{i}") for i in range(2)]
                pcs = [small.tile([128, 2], F32, tag=f"pcs{i}", name=f"pcs{i}") for i in range(2)]
                for i in range(2):
                    stats = work.tile([128, 8, 6], F32, tag="bnstats", name="bnstats")
                    for j in range(8):
                        nc.vector.bn_stats(stats[:, j, :], x[i][:, 512 * j:512 * (j + 1)])
                    mv = work.tile([128, 2], F32, tag="bnmv", name="bnmv")
                    nc.vector.bn_aggr(mv[:], stats[:])
                    # pcs = (mean, E[x^2]) per channel
                    nc.vector.tensor_copy(pcs[i][:, 0:1], mv[:, 0:1])
                    nc.vector.tensor_tensor(pcs[i][:, 1:2], mv[:, 0:1], mv[:, 0:1],
                                            mybir.AluOpType.mult)
                    nc.vector.tensor_tensor(pcs[i][:, 1:2], pcs[i][:, 1:2], mv[:, 1:2],
                                            mybir.AluOpType.add)

                # group stats [G, 2] = (mean_g, E[x^2]_g)
                grp_ps = qk_ps()[:G, 0, 0:2]
                nc.tensor.matmul(grp_ps, gsel[:, 0:G], pcs[0][:], start=True, stop=False)
                nc.tensor.matmul(grp_ps, gsel[:, G:2 * G], pcs[1][:], start=False, stop=True)

                grp_sb = small.tile([G, 2], F32, tag="grp_sb", name="grp_sb")
                nc.vector.tensor_copy(grp_sb[:], grp_ps)
                grp2 = small.tile([G, 2], F32, tag="grp2", name="grp2")  # (mean, rstd)
                var = small.tile([G, 1], F32, tag="var", name="var")
                nc.vector.tensor_copy(grp2[:, 0:1], grp_sb[:, 0:1])
                nc.vector.tensor_tensor(var[:], grp_sb[:, 0:1], grp_sb[:, 0:1],
                                        mybir.AluOpType.mult)
                nc.vector.tensor_tensor(var[:], grp_sb[:, 1:2], var[:],
                                        mybir.AluOpType.subtract)
                nc.vector.tensor_scalar(var[:], var[:], EPS, None,
                                        mybir.AluOpType.add)
                # rstd = rsqrt(var+eps) fully on DVE: magic-number seed + two
                # Newton iterations (no activation tables on the startup path)
                yr = small.tile([G, 1], F32, tag="yr", name="yr")
                t1 = small.tile([G, 1], mybir.dt.int32, tag="t1", name="t1")
                nc.vector.tensor_scalar(t1[:], var[:].bitcast(mybir.dt.int32),
                                        1, None,
                                        mybir.AluOpType.logical_shift_right)
                nc.vector.tensor_scalar(t1[:], t1[:], 0x5f3759df, -1,
                                        mybir.AluOpType.subtract,
                                        mybir.AluOpType.mult)
                nc.vector.tensor_copy(yr[:].bitcast(mybir.dt.int32), t1[:])
                ytmp = small.tile([G, 1], F32, tag="ytmp", name="ytmp")
                for _ in range(2):
                    nc.vector.tensor_tensor(ytmp[:], yr[:], yr[:],
                                            mybir.AluOpType.mult)
                    nc.vector.tensor_tensor(ytmp[:], ytmp[:], var[:],
                                            mybir.AluOpType.mult)
                    nc.vector.tensor_scalar(ytmp[:], ytmp[:], -0.5, 1.5,
                                            mybir.AluOpType.mult,
                                            mybir.AluOpType.add)
                    nc.vector.tensor_tensor(yr[:], yr[:], ytmp[:],
                                            mybir.AluOpType.mult)
                nc.vector.tensor_copy(grp2[:, 1:2], yr[:])

                # broadcast to channels; A = rstd*gn_w, B = gn_b - mean*A
                for i in range(2):
                    mb_ps = qk_ps()[:, 0, 0:2]
                    nc.tensor.matmul(mb_ps, bsel[:, 128 * i:128 * (i + 1)], grp2[:],
                                     start=True, stop=True)
                    nc.vector.tensor_tensor(A[i][:], mb_ps[:, 1:2], gnw[i][:],
                                            mybir.AluOpType.mult)
                    nc.vector.tensor_tensor(Bs[i][:], mb_ps[:, 0:1], A[i][:],
                                            mybir.AluOpType.mult)
                    nc.vector.tensor_tensor(Bs[i][:], gnb[i][:], Bs[i][:],
                                            mybir.AluOpType.subtract)
                    # folded weights
                    nc.vector.tensor_scalar_mul(wtf[i][:], wt[i][:], A[i][:])

                # folded q/k biases: qkvb[o] + sum_c wT[c,o]*B[c].  Only the
                # pair-0 biases (o=0, o=2) fold up front; o=1/o=3 are deferred
                # into the attention stream (their epilogues run much later).
                def fold_qkb(o):
                    b_ps = qk_ps()[:, 0, 0:1]
                    nc.tensor.matmul(b_ps, wt[0][:, 128 * o:128 * (o + 1)], Bs[0][:],
                                     start=True, stop=False)
                    nc.tensor.matmul(b_ps, wt[1][:, 128 * o:128 * (o + 1)], Bs[1][:],
                                     start=False, stop=True)
                    nc.vector.tensor_tensor(qkb[o][:], qkb[o][:], b_ps,
                                            mybir.AluOpType.add)

                def fold_vb():
                    vb_ps = qk_ps()[0:1, 0, 0:C]
                    nc.tensor.matmul(vb_ps, Bs[0][:], wt[0][:, 2 * C:3 * C],
                                     start=True, stop=False)
                    nc.tensor.matmul(vb_ps, Bs[1][:], wt[1][:, 2 * C:3 * C],
                                     start=False, stop=True)
                    vb_row = small.tile([1, C], F32, tag="vb_row", name="vb_row")
                    nc.vector.tensor_tensor(vb_row[:], vb_ps, vbrow[:],
                                            mybir.AluOpType.add)
                    nc.gpsimd.partition_broadcast(vbias_bc[:], vb_row[:])

                nc.gpsimd.memset(vt[:, :, :, CH:CH + 1], 1.0)  # ones col (denominator)
                nc.gpsimd.memset(vt[:, :, :, CH + 1:2 * CH], 0.0)  # zero pad

                # ================= production helpers =====================
                def qk_pair(dst, wofs, bias, o, t, ps=None):
                    # two 512-col t-tiles of q or k output block o.  Queries
                    # read x cols 0..2047 directly (host pre-rotated).
                    ts2 = slice(512 * t, 512 * (t + 2))
                    if ps is None:
                        ps = pp_tile()
                    for u in range(2):
                        ts = slice(512 * (t + u), 512 * (t + u + 1))
                        nc.tensor.matmul(ps[:, u, :],
                                         wtf[0][:, wofs + 128 * o:wofs + 128 * (o + 1)],
                                         x[0][:, ts], start=True, stop=False)
                        nc.tensor.matmul(ps[:, u, :],
                                         wtf[1][:, wofs + 128 * o:wofs + 128 * (o + 1)],
                                         x[1][:, ts], start=False, stop=True)
                    nc.vector.tensor_scalar(dst[o][:, ts2].rearrange("p (u f) -> p u f", u=2),
                                            ps[:], bias[:], SCALE,
                                            mybir.AluOpType.add, mybir.AluOpType.mult)

                def vt_quad(sc, ps=None):
                    # four 128-wide s-chunks through one pp slot
                    ps = (pp_tile() if ps is None else ps).rearrange(
                        "p u (v f) -> p (u v) f", v=2)
                    for u in range(4):
                        ss = slice(128 * (sc + u), 128 * (sc + u + 1))
                        nc.tensor.matmul(ps[:, u, :], x[0][:, ss], wtf[0][:, 2 * C:3 * C],
                                         start=True, stop=False)
                        nc.tensor.matmul(ps[:, u, :], x[1][:, ss], wtf[1][:, 2 * C:3 * C],
                                         start=False, stop=True)
                    nc.vector.tensor_tensor(
                        vt[:, sc:sc + 4, :, 0:CH],
                        ps.rearrange("p u (h c) -> p u h c", h=NH),
                        bass.AP(tensor=vbias_bc[:].tensor, offset=vbias_bc[:].offset,
                                ap=[vbias_bc[:].ap[0], [0, 4], [CH, NH], [1, CH]]),
                        mybir.AluOpType.add)

                # minimal pre-attention production: q/k for pair 0's start +
                # the first vt quad, spread over three psum slots so the
                # matmuls run back-to-back instead of serializing on one pool.
                # Bias folds interleave so each epilogue's bias lands in time.
                fold_qkb(0)
                qk_pair(q_sb, 0, qkb[0], 0, 0)
                fold_qkb(2)
                qk_pair(k_sb, C, qkb[2], 0, 0, ps=qk_ps())
                fold_vb()
                vt_quad(0, ps=qk_ps())

                # deferred production, keyed by global group index
                prod_at = {}

                def at(g, fn):
                    prod_at.setdefault(g, []).append(fn)

                # vt quads 1..7 at groups 4j+1 (deadline: AV(4j) at 4j+LAG)
                for j in range(1, 8):
                    at(4 * j - 2, lambda j=j: vt_quad(4 * j))
                # k pair t covers s-chunks 4t..4t+7; QK needs it at group 4t
                at(5, lambda: qk_pair(k_sb, C, qkb[2], 0, 2))
                at(13, lambda: qk_pair(k_sb, C, qkb[2], 0, 4))
                at(21, lambda: qk_pair(k_sb, C, qkb[2], 0, 6))
                # remaining q/k for later units (unit u starts at group 32u)
                # both deferred bias folds together: two back-to-back psQK
                # allocations keep the qkp double-buffer parity unchanged
                at(30, lambda: (fold_qkb(3), fold_qkb(1)))
                at(33, lambda: qk_pair(q_sb, 0, qkb[0], 0, 2))
                at(36, lambda: qk_pair(k_sb, C, qkb[3], 1, 0))
                at(40, lambda: qk_pair(k_sb, C, qkb[3], 1, 2))
                at(45, lambda: qk_pair(k_sb, C, qkb[3], 1, 4))
                at(48, lambda: qk_pair(k_sb, C, qkb[3], 1, 6))
                at(52, lambda: qk_pair(q_sb, 0, qkb[1], 1, 0))
                at(56, lambda: qk_pair(q_sb, 0, qkb[1], 1, 2))

                # ================= attention + proj =======================
                with tc.tile_pool(name="pexp", bufs=LAG + 2) as pexp, \
                     tc.tile_pool(name="nrm", bufs=4) as nrm, \
                     tc.tile_pool(name="outp", bufs=3) as outp:

                    def emit_av(acc, p, sc, pe):
                        for j in range(2):
                            h = 2 * p + j
                            nc.tensor.matmul(acc[:, j, :], vt[:, sc, h, :],
                                             pe[:, j, :],
                                             start=(sc == 0), stop=(sc == NSC - 1))

                    def normalize(p, tb, acc):
                        # reciprocal reads the denominator row straight from
                        # PSUM; the two heads' chains interleave so the DVE
                        # and Pool stages pipeline instead of serializing.
                        tbs = slice(512 * tb, 512 * (tb + 1))
                        den = [nrm.tile([1, 512], F32, tag=f"den{hi}", name=f"den{hi}")
                               for hi in range(2)]
                        rec = [nrm.tile([1, 512], F32, tag=f"rec{hi}", name=f"rec{hi}")
                               for hi in range(2)]
                        bc = [nrm.tile([CH, 512], F32, tag=f"bc{hi}", name=f"bc{hi}")
                              for hi in range(2)]
                        for hi in range(2):
                            nc.vector.tensor_copy(den[hi][:], acc[CH:CH + 1, hi, :])
                        for hi in range(2):
                            nc.vector.reciprocal_approx_fast(rec[hi][:], den[hi][:])
                        for hi in range(2):
                            nc.gpsimd.partition_broadcast(bc[hi][:], rec[hi][:])
                        for hi in range(2):
                            hp = slice(64 * hi, 64 * hi + 64)
                            nc.vector.tensor_tensor(a_sb[p][hp, tbs],
                                                    acc[0:CH, hi, :],
                                                    bc[hi][:], mybir.AluOpType.mult)

                    def proj_tb(tb):
                        tbs = slice(512 * tb, 512 * (tb + 1))
                        pr = pp_tile()
                        for o in range(2):
                            nc.tensor.matmul(pr[:, o, :], pjt[0][:, 128 * o:128 * (o + 1)],
                                             a_sb[0][:, tbs], start=True, stop=False)
                            nc.tensor.matmul(pr[:, o, :], pjt[1][:, 128 * o:128 * (o + 1)],
                                             a_sb[1][:, tbs], start=False, stop=True)
                        for o in range(2):
                            res = outp.tile([128, 512], F32, tag="res", name="res")
                            nc.vector.tensor_scalar(res[:], pr[:, o, :], pjb[o][:], None,
                                                    mybir.AluOpType.add)
                            nc.vector.tensor_tensor(res[:], res[:],
                                                    xqf[o][:, tbs],
                                                    mybir.AluOpType.add)
                            nc.sync.dma_start(d_out[128 * o:128 * (o + 1), tbs], res[:])

                    pend = deque()   # (acc, p, sc, pe, post)
                    units = [(p, tb) for p in range(2) for tb in range(THALF // 512)]
                    gg = 0
                    for (p, tb) in units:
                        tbs = slice(512 * tb, 512 * (tb + 1))
                        acc = psA.tile([128, 2, 512], F32, tag="acc", name="acc")
                        for sc in range(NSC):
                            qkp = psQK.tile([128, 2, 512], F32, tag="qk", name="qk")
                            for j in range(2):
                                hp = slice(64 * j, 64 * j + 64)
                                nc.tensor.matmul(
                                    qkp[:, j, :],
                                    k_sb[p][hp, 128 * sc:128 * (sc + 1)],
                                    q_sb[p][hp, tbs],
                                    start=True, stop=True)
                            pe = pexp.tile([128, 2, 512], P_DT, tag="pe", name="pe")
                            nc.scalar.activation(pe[:], qkp[:],
                                                 mybir.ActivationFunctionType.Exp)
                            for fn in prod_at.pop(gg, []):
                                fn()
                            post = None
                            if sc == NSC - 1:
                                def post(p=p, tb=tb, acc=acc):
                                    normalize(p, tb, acc)
                                    if p == 1:
                                        proj_tb(tb)
                            pend.append((acc, p, sc, pe, post))
                            # taper-drain: pop 2 per group near the unit end so
                            # the lag is zero at the boundary and normalize can
                            # free the acc banks before the next unit's AVs.
                            npop = 2 if sc >= NSC - LAG else (
                                1 if len(pend) > LAG else 0)
                            for _ in range(npop):
                                if not pend:
                                    break
                                ent = pend.popleft()
                                emit_av(*ent[:4])
                                if ent[4] is not None:
                                    ent[4]()
                            gg += 1
                    while pend:
                        ent = pend.popleft()
                        emit_av(*ent[:4])
                        if ent[4] is not None:
                            ent[4]()

    nc.compile()
    return nc


def _host_consts():
    g1 = np.zeros((128, G), dtype=np.float32)
    g2 = np.zeros((128, G), dtype=np.float32)
    for c in range(128):
        g1[c, c // CPG] = 1.0 / CPG
        g2[c, G // 2 + c // CPG] = 1.0 / CPG
    gsel = np.concatenate([g1, g2], axis=1)          # [128, 2G]
    bsel = np.zeros((G, C), dtype=np.float32)
    for c in range(C):
        bsel[c // CPG, c] = 1.0
    return gsel, bsel


def kernel(x, gn_w, gn_b, qkv_w, qkv_b, proj_w, proj_b):
    global LAST_RESULTS
    if "nc" not in _CACHE:
        _CACHE["nc"] = _build_program()
    nc = _CACHE["nc"]

    x = np.ascontiguousarray(np.asarray(x, dtype=np.float32))
    xr = x.reshape(B, C, T)
    gsel, bsel = _host_consts()
    shared = {
        "wT": np.ascontiguousarray(np.asarray(qkv_w, np.float32).T),
        "qkvb": np.asarray(qkv_b, np.float32).reshape(3 * C, 1).copy(),
        "vbrow": np.asarray(qkv_b, np.float32)[2 * C:].reshape(1, C).copy(),
        "pjT": np.ascontiguousarray(np.asarray(proj_w, np.float32).T.astype(np.float16)),
        "pjb": np.asarray(proj_b, np.float32).reshape(C, 1).copy(),
        "gnw": np.asarray(gn_w, np.float32).reshape(C, 1).copy(),
        "gnb": np.asarray(gn_b, np.float32).reshape(C, 1).copy(),
        "gsel": gsel,
        "bsel": bsel,
    }
    in_maps = []
    for c in range(NCORES):
        b, hf = c // 2, c % 2
        m = dict(shared)
        # rotate so this core's query half is always columns 0..2047
        xrot = np.roll(xr[b], -hf * THALF, axis=1)
        m["xb"] = np.ascontiguousarray(xrot.astype(np.float16))
        m["xqf"] = np.ascontiguousarray(xrot[:, :THALF])
        in_maps.append(m)

    res = bass_utils.run_bass_kernel_spmd(nc, in_maps, core_ids=list(range(NCORES)))
    LAST_RESULTS = res

    out = np.empty((B, C, T), dtype=np.float32)
    for c in range(NCORES):
        b, hf = c // 2, c % 2
        out[b][:, hf * THALF:(hf + 1) * THALF] = res.results[c]["out"]
    return out.reshape(B, C, HH, WW)
